# revision 1
# baseline (speedup 1.0000x reference)
"""Trainium2 Bass kernel for nn_CNN_BiMACL_31860067401819 (retrieval_knn).

Self-contained: hardcodes all shapes/sharding. kernel(**inputs) accepts FULL
inputs keyed as in setup_inputs(), shards queries across 8 NeuronCores
(data-parallel over the query axis), and returns the FULL [2, 320, 5] f32
output. The only collective is a tiny AllReduce of the per-class `rec`
statistics (which couple all queries in the reference).

Per-core query-tuple row order is i = t*40 + q (t-major); per-query means are
recovered with a small constant selection matmul (Sel). Support-tuple
embeddings are written permuted to class-major order (c,sh,t) during the
PSUM->SBUF relu pass.
"""
import os
from itertools import combinations

import numpy as np

import concourse.bass as bass
import concourse.tile as tile
from concourse import bacc, mybir
from concourse.bass_utils import run_bass_kernel_spmd

# ---- static problem config ----
WAY, SHOT, SEQ_LEN, TSS = 5, 16, 10, 2
DIN, DOUT = 2048, 1152
N_QUERIES = 320
T = 45
S = SHOT * T                 # 720
SALL = WAY * S               # 3600
NCORES = 8
NQ = N_QUERIES // NCORES     # 40
R = NQ * T                   # 1800 valid rows/core
RHAT = 1920                  # 48 t-slots * 40 q = 15*128
ITILES = RHAT // 128         # 15
K = TSS * DIN                # 4096
KC = K // 128                # 32
DC = DOUT // 128             # 9
TUPLES = np.array(list(combinations(range(SEQ_LEN), TSS)), dtype=np.int32)
SPAD = 3712                  # padded sup cols (29*128)
PTILES = 29
PROW = 3712                  # p_dram row pitch in bf16 elems (bytes % 256 == 0)
SC = 450                     # D/SS matmul free-dim chunk
NSC = SALL // SC             # 8
QIC = 480                    # q emb row chunk = 12 t-groups of 40
NQIC = RHAT // QIC           # 4
SIC = 480                    # sup emb row chunk = 6 t-groups of 80
SHAT = 48 * 80               # 3840 padded sup rows, (t,u) order
NSIC = SHAT // SIC           # 8

F32 = mybir.dt.float32
BF16 = mybir.dt.bfloat16
U32 = mybir.dt.uint32
I16 = mybir.dt.int16

_CACHE = {}


def _ap(tensor, offset, dims):
    return bass.AP(tensor=tensor, offset=offset, ap=[list(d) for d in dims])


def build(debug=False, sim1=False, stop_after=None):
    """Build the per-core program. sim1: replace the AllReduce with a local
    copy so single-core simulators can run it."""
    nc = bacc.Bacc(num_swdge_queues=4)
    q_d = nc.dram_tensor("qT", [128, 16, NQ * SEQ_LEN], BF16, kind="ExternalInput")
    s_d = nc.dram_tensor("sT", [128, 16, 80 * SEQ_LEN], BF16, kind="ExternalInput")
    w_d = nc.dram_tensor("wT", [KC, 128, DOUT], BF16, kind="ExternalInput")
    b_d = nc.dram_tensor("b", [DOUT], F32, kind="ExternalInput")
    sel_d = nc.dram_tensor("sel", [ITILES, 128, NQ], F32, kind="ExternalInput")
    padv_d = nc.dram_tensor("padv", [128, 1], F32, kind="ExternalInput")
    out_d = nc.dram_tensor("out", [2, NQ, WAY], F32, kind="ExternalOutput")
    dbg = {}
    if debug:
        dbg["D"] = nc.dram_tensor("dbg_D", [RHAT, SALL], F32, kind="ExternalOutput")
        dbg["ave"] = nc.dram_tensor("dbg_ave", [128, ITILES, WAY], F32, kind="ExternalOutput")
        dbg["pos"] = nc.dram_tensor("dbg_pos", [128, ITILES, WAY], F32, kind="ExternalOutput")
        dbg["rec"] = nc.dram_tensor("dbg_rec", [WAY, SALL], F32, kind="ExternalOutput")
        dbg["mask"] = nc.dram_tensor("dbg_mask", [WAY, SALL], F32, kind="ExternalOutput")
        dbg["dmax"] = nc.dram_tensor("dbg_dmax", [128, ITILES, WAY], F32, kind="ExternalOutput")
        dbg["semb"] = nc.dram_tensor("dbg_semb", [128, DC, SPAD], F32, kind="ExternalOutput")

    with tile.TileContext(nc) as tc:
        _body(nc, tc, q_d, s_d, w_d, b_d, sel_d, padv_d, out_d, dbg, sim1, stop_after)
    nc.finalize()
    return nc


def _body(nc, tc, q_d, s_d, w_d, b_d, sel_d, padv_d, out_d, dbg, sim1, stop_after):
    AT = mybir.AluOpType
    ACTF = mybir.ActivationFunctionType
    X = mybir.AxisListType.X

    persist = tc.alloc_tile_pool(name="persist", bufs=1)
    dram = tc.alloc_tile_pool(name="dram", bufs=1, space="DRAM")

    # DRAM scratch (pool tiles so Tile tracks cross-phase deps)
    p_dram = dram.tile([SALL, PROW], BF16, tag="p_scratch")
    qembT_dram = dram.tile([DC, 128, RHAT], BF16, tag="qembT")
    dbf_dram = dram.tile([RHAT, SALL], BF16, tag="dbf")
    posw_dram = dram.tile([WAY, 16, ITILES * 8], I16, tag="posw")
    snorm_dram = dram.tile([1, SALL], F32, tag="snormd")
    mask_dram = dram.tile([WAY, SALL], BF16, tag="maskd")
    msum_dram = dram.tile([WAY, 1], F32, tag="msumd")
    cc_in = dram.tile([WAY, SALL], F32, tag="cc_in")
    cc_out = dram.tile([WAY, SALL], F32, tag="cc_out")

    # persistent SBUF (whole-kernel lifetime -- keep this SMALL)
    s_embT = persist.tile([128, DC, SPAD], BF16, tag="s_embT")
    ones_col = persist.tile([128, 1], BF16, tag="ones_col")
    nc.vector.memset(ones_col[:], 1.0)
    onesf_col = persist.tile([128, 1], F32, tag="onesf_col")
    nc.vector.memset(onesf_col[:], 1.0)
    qnorm = persist.tile([128, ITILES], F32, tag="qnorm")
    pnorm = persist.tile([128, PTILES], F32, tag="pnorm")
    ave_all = persist.tile([128, ITILES, WAY], F32, tag="ave_all")
    dmax_all = persist.tile([128, ITILES, WAY], F32, tag="dmax_all")
    pos16 = persist.tile([128, ITILES, WAY], I16, tag="pos16")
    msum = persist.tile([WAY, 1], F32, tag="msum")
    padv = persist.tile([128, 1], F32, tag="padv")
    nc.sync.dma_start(padv[:], padv_d[:, :])
    rowacc = persist.tile([128, ITILES, WAY], F32, tag="rowacc")
    dmaxq = persist.tile([1, WAY, NQ], F32, tag="dmaxq")
    ctq = persist.tile([1, WAY, NQ], F32, tag="ctq")

    nc.vector.memset(s_embT[:, :, SALL:SPAD], 0.0)

    # ================= Phase 1: embeddings =================
    # Host supplies wT/qT/sT already bf16 + transposed (k on partitions).
    with tc.tile_pool(name="emb", bufs=1) as emb, \
         tc.tile_pool(name="embg", bufs=2) as embg, \
         tc.tile_pool(name="embps", bufs=4, space="PSUM") as embps:
        wT = emb.tile([128, KC, DOUT], BF16, tag="wT")
        nc.sync.dma_start(wT[:], w_d.rearrange("kc p d -> p kc d"))

        # ---- q-side: gather xg from DRAM qT; spill embT to DRAM ----
        for ic in range(NQIC):
            xg = embg.tile([128, KC, QIC], BF16, tag="xg")
            t0 = ic * 12
            if t0 + 12 > T:
                nc.vector.memset(xg[:, :, (T - t0) * 40:], 0.0)
            for tl in range(min(12, T - t0)):
                t = t0 + tl
                for h in range(2):
                    fr = int(TUPLES[t][h])
                    nc.sync.dma_start(
                        xg[:, h * 16:(h + 1) * 16, tl * 40:(tl + 1) * 40],
                        q_d[:, :, fr * 40:(fr + 1) * 40])
            for dc in range(DC):
                ps = embps.tile([128, QIC], F32, tag="emb_ps")
                for kc in range(KC):
                    nc.tensor.matmul(ps[:], wT[:, kc, dc * 128:(dc + 1) * 128],
                                     xg[:, kc], start=(kc == 0), stop=True)
                qe = embg.tile([128, QIC], BF16, tag="qe")
                nc.scalar.activation(qe[:], ps[:], ACTF.Relu)
                nc.sync.dma_start(
                    qembT_dram[dc, :, ic * QIC:(ic + 1) * QIC], qe[:])

        # ---- sup-side: gather xg from SBUF sT; permuted relu writes ----
        for ic in range(NSIC):
            xg = embg.tile([128, KC, SIC], BF16, tag="xg")
            t0 = ic * 6
            if t0 + 6 > T:
                nc.vector.memset(xg[:, :, (T - t0) * 80:], 0.0)
            for tl in range(min(6, T - t0)):
                t = t0 + tl
                for h in range(2):
                    fr = int(TUPLES[t][h])
                    nc.sync.dma_start(
                        xg[:, h * 16:(h + 1) * 16, tl * 80:(tl + 1) * 80],
                        s_d[:, :, fr * 80:(fr + 1) * 80])
            for dc in range(DC):
                ps = embps.tile([128, SIC], F32, tag="emb_ps")
                for kc in range(KC):
                    nc.tensor.matmul(ps[:], wT[:, kc, dc * 128:(dc + 1) * 128],
                                     xg[:, kc], start=(kc == 0), stop=True)
                for tl in range(min(6, T - t0)):
                    t = t0 + tl
                    dstp = s_embT[:, dc, :SALL].rearrange(
                        "p (u t) -> p t u", t=T)[:, t]
                    nc.scalar.activation(dstp, ps[:, tl * 80:(tl + 1) * 80],
                                         ACTF.Relu)

    # ================= Phase 2: reload q_embT, norms, SS, D ================
    ph2 = tc.alloc_tile_pool(name="ph2", bufs=1)
    q_embT = ph2.tile([128, DC, RHAT], BF16, tag="q_embT")
    nc.sync.dma_start(q_embT[:], qembT_dram.rearrange("d p i -> p d i"))

    def _stop(tag):
        if stop_after == tag:
            ph2.release(); persist.release(); dram.release()
            return True
        return False

    if _stop("emb"):
        return

    if dbg:
        with tc.tile_pool(name="dbge", bufs=1) as dbge:
            t2 = dbge.tile([128, DC, SPAD], F32, tag="t2")
            nc.vector.tensor_copy(t2[:], s_embT[:])
            nc.sync.dma_start(dbg["semb"].ap(), t2[:])

    # ---- norms ----
    with tc.tile_pool(name="nrm", bufs=2) as nrm, \
         tc.tile_pool(name="nrmps", bufs=2, space="PSUM") as nrmps:
        snorm_row = nrm.tile([1, SALL], F32, tag="snorm_row")
        for (nt, src_t, dst_t) in ((ITILES, q_embT, qnorm), (PTILES, s_embT, pnorm)):
            for it in range(nt):
                ps = nrmps.tile([128, 1], F32, tag="qn_ps", name="qnps")
                sqa = nrm.tile([128, DC, 128], BF16, tag="qn_sqb")
                for dc in range(DC):
                    nc.scalar.activation(sqa[:, dc],
                                         src_t[:, dc, it * 128:(it + 1) * 128],
                                         ACTF.Square)
                for dc in range(DC):
                    nc.tensor.matmul(ps[:], sqa[:, dc], ones_col[:],
                                     start=(dc == 0), stop=(dc == DC - 1))
                nc.vector.tensor_copy(dst_t[:, it:it + 1], ps[:])
        for scn in range(8):
            ps = nrmps.tile([1, 450], F32, tag="sn_ps")
            for dc in range(DC):
                sq = nrm.tile([128, 450], BF16, tag="sn_sqb")
                nc.scalar.activation(sq[:], s_embT[:, dc, scn * 450:(scn + 1) * 450],
                                     ACTF.Square)
                nc.tensor.matmul(ps[:], ones_col[:], sq[:],
                                 start=(dc == 0), stop=(dc == DC - 1))
            nc.vector.tensor_copy(snorm_row[:, scn * 450:(scn + 1) * 450], ps[:])
        nc.sync.dma_start(snorm_dram[:, :], snorm_row[:])

    if _stop("norms"):
        return

    # ---- SS all-pairs -> p_dram, then D + reductions ----
    with tc.tile_pool(name="snb", bufs=1) as snb, \
         tc.tile_pool(name="ssp", bufs=2) as ssp, \
         tc.tile_pool(name="ssps", bufs=1, space="PSUM") as ssps:
        snorm_bc = snb.tile([128, SALL], F32, tag="snorm_bc")
        nc.sync.dma_start(snorm_bc[:], _ap(snorm_dram.tensor, snorm_dram.offset,
                                           [(0, 128), (1, SALL)]))
        for pt in range(PTILES):
            prow = min(128, SALL - pt * 128)
            ss_sb = ssp.tile([128, SALL], F32, tag="ss_sb")
            pss = [ssps.tile([128, SC], F32, tag=f"ss_ps{sc}", name=f"ssps{sc}") for sc in range(NSC)]
            for dc in range(DC):
                for sc in range(NSC):
                    nc.tensor.matmul(pss[sc][:],
                                     s_embT[:, dc, pt * 128:(pt + 1) * 128],
                                     s_embT[:, dc, sc * SC:(sc + 1) * SC],
                                     start=(dc == 0), stop=(dc == DC - 1))
            for sc in range(NSC):
                dst = ss_sb[:, sc * SC:(sc + 1) * SC]
                nc.vector.scalar_tensor_tensor(
                    dst, pss[sc][:], -2.0, snorm_bc[:, sc * SC:(sc + 1) * SC],
                    op0=AT.mult, op1=AT.add)
                if (sc * SC < (pt + 1) * 128) and ((sc + 1) * SC > pt * 128):
                    nc.vector.tensor_scalar(dst, dst, pnorm[:, pt:pt + 1], 1e-12,
                                            AT.add, op1=AT.max)
                    nc.scalar.activation(dst, dst, ACTF.Sqrt)
                else:
                    nc.scalar.activation(dst, dst, ACTF.Sqrt,
                                         bias=pnorm[:, pt:pt + 1])
            ss_bf = ssp.tile([128, SALL], BF16, tag="ss_bf")
            nc.vector.tensor_copy(ss_bf[:], ss_sb[:])
            nc.sync.dma_start(
                _ap(p_dram.tensor, p_dram.offset + pt * 128 * PROW,
                    [(PROW, prow), (1, SALL)]),
                ss_bf[:prow])

        with tc.tile_pool(name="dp", bufs=2) as dp, \
             tc.tile_pool(name="redt", bufs=4) as redt, \
             tc.tile_pool(name="cdp", bufs=1) as cdp, \
             tc.tile_pool(name="cdg", bufs=2) as cdg:
            SC2 = 360
            for c in range(WAY):
                for it in range(ITILES):
                    d_sb = dp.tile([128, S], F32, tag="d_sb")
                    pss = [ssps.tile([128, SC], F32,
                                     tag=f"ss_ps{(it % 2) * 2 + sc}",
                                     name=f"dps{sc}")
                           for sc in range(2)]
                    for dc in range(DC):
                        for sc in range(2):
                            nc.tensor.matmul(
                                pss[sc][:, :SC2],
                                q_embT[:, dc, it * 128:(it + 1) * 128],
                                s_embT[:, dc, c * S + sc * SC2:c * S + (sc + 1) * SC2],
                                start=(dc == 0), stop=(dc == DC - 1))
                    for sc in range(2):
                        dst = d_sb[:, sc * SC2:(sc + 1) * SC2]
                        nc.vector.scalar_tensor_tensor(
                            dst, pss[sc][:, :SC2], -2.0,
                            snorm_bc[:, c * S + sc * SC2:c * S + (sc + 1) * SC2],
                            op0=AT.mult, op1=AT.add)
                        nc.scalar.activation(dst, dst, ACTF.Sqrt,
                                             bias=qnorm[:, it:it + 1])
                    if dbg:
                        nc.sync.dma_start(
                            dbg["D"][it * 128:(it + 1) * 128, c * S:(c + 1) * S],
                            d_sb[:])
                    d_bf = dp.tile([128, S], BF16, tag="d_bf")
                    nc.vector.tensor_copy(d_bf[:], d_sb[:])
                    nc.sync.dma_start(
                        dbf_dram[it * 128:(it + 1) * 128, c * S:(c + 1) * S],
                        d_bf[:])
                    m16 = redt.tile([128, 16], F32, tag="m16")
                    nc.vector.tensor_reduce(
                        m16[:], d_sb[:].rearrange("p (a b) -> p b a", b=16),
                        X, AT.max)
                    asum = redt.tile([128, 1], F32, tag="asum")
                    nc.vector.tensor_reduce(asum[:], m16[:], X, AT.add)
                    nc.vector.tensor_scalar(ave_all[:, it, c:c + 1], asum[:],
                                            1.0 / 16.0, None, AT.mult)
                    nc.vector.tensor_reduce(dmax_all[:, it, c:c + 1], m16[:],
                                            X, AT.max)
                    mx8 = redt.tile([128, 8], F32, tag="mx8")
                    ix8 = redt.tile([128, 8], U32, tag="ix8")
                    nc.vector.max(mx8[:], d_sb[:])
                    nc.vector.max_index(ix8[:], mx8[:], d_sb[:])
                    posf = redt.tile([128, 1], F32, tag="posf")
                    nc.vector.tensor_scalar(posf[:], ix8[:, 0:1], float(c * S),
                                            None, AT.add)
                    nc.vector.tensor_copy(pos16[:, it, c:c + 1], posf[:])
                    if it == ITILES - 1:
                        nc.vector.tensor_scalar(
                            ave_all[:, it, c:c + 1], ave_all[:, it, c:c + 1],
                            padv[:], None, AT.add)
                # ---- CD gather + rec for class c (overlaps next class's D) --
                nc.sync.dma_start(
                    _ap(posw_dram.tensor, posw_dram.offset + c * 16 * ITILES * 8,
                        [(1, 8), (ITILES * 8, 16), (8, ITILES)]),
                    pos16[:, :, c])
                idxs = cdp.tile([128, ITILES * 8], I16, tag="idxs")
                nc.sync.dma_start(
                    idxs[:],
                    _ap(posw_dram.tensor, posw_dram.offset + c * 16 * ITILES * 8,
                        [(0, 8), (ITILES * 8, 16), (1, ITILES * 8)]))
                acc = cdp.tile([128, SALL], F32, tag="acc01")
                nc.vector.memset(acc[:], 0.0)
                for g in range(ITILES):
                    cd = cdg.tile([128, 1, PROW], BF16, tag="cd")
                    nc.gpsimd.dma_gather(
                        cd[:], p_dram[:, :], idxs[:, g * 8:(g + 1) * 8],
                        128, 128, PROW, queue_num=g % 4)
                    nc.vector.scalar_tensor_tensor(
                        acc[:], cd[:, 0, :SALL], ave_all[:, g, c:c + 1], acc[:],
                        op0=AT.is_gt, op1=AT.add)
                for ch in range(8):
                    ps = ssps.tile([1, 450], F32, tag=f"ss_ps{2 + (ch % 6)}",
                                   name=f"recps{ch}")
                    nc.tensor.matmul(ps[:], onesf_col[:],
                                     acc[:, ch * 450:(ch + 1) * 450],
                                     start=True, stop=True)
                    rc_sb = cdg.tile([1, 450], F32, tag="rec_sb")
                    nc.scalar.copy(rc_sb[:], ps[:])
                    nc.sync.dma_start(cc_in[c:c + 1, ch * 450:(ch + 1) * 450],
                                      rc_sb[:])

    if dbg:
        nc.sync.dma_start(dbg["ave"].ap(), ave_all[:])
        nc.sync.dma_start(dbg["dmax"].ap(), dmax_all[:])
        with tc.tile_pool(name="dbgp", bufs=1) as dbgp:
            pf = dbgp.tile([128, ITILES, WAY], F32, tag="pf")
            nc.vector.tensor_copy(pf[:], pos16[:])
            nc.sync.dma_start(dbg["pos"].ap(), pf[:])

    if _stop("ssd"):
        return

    if _stop("gather"):
        return

    # ================= AllReduce rec =================
    if sim1:
        nc.sync.dma_start(cc_out[:, :], cc_in[:, :])
    else:
        nc.gpsimd.collective_compute(
            "AllReduce", AT.add, replica_groups=[list(range(NCORES))],
            ins=[cc_in[:, :].opt()], outs=[cc_out[:, :].opt()])

    # ================= Phase 3: thr/mask (base-0 partition ops only) =======
    with tc.tile_pool(name="thrp", bufs=2) as thrp, \
         tc.tile_pool(name="thrbig", bufs=1) as thrbig:
        rec_slots = thrbig.tile([WAY, WAY - 1, S], F32, tag="rec_slots")
        for c in range(WAY):
            for k in range(WAY - 1):
                oc = k if k < c else k + 1
                nc.sync.dma_start(rec_slots[c:c + 1, k],
                                  cc_out[c:c + 1, oc * S:(oc + 1) * S])
        if dbg:
            with tc.tile_pool(name="dbgr", bufs=1) as dbgr:
                rg = dbgr.tile([WAY, SALL], F32, tag="rg")
                nc.sync.dma_start(rg[:], cc_out[:, :])
                nc.sync.dma_start(dbg["rec"].ap(), rg[:])
        rsum = thrp.tile([WAY, WAY - 1], F32, tag="rsum")
        nc.vector.tensor_reduce(rsum[:], rec_slots[:], X, AT.add)
        gt0 = thrbig.tile([WAY, WAY - 1, S], F32, tag="gt0")
        nc.vector.tensor_scalar(gt0[:], rec_slots[:], 0.0, None, AT.is_gt)
        nz = thrp.tile([WAY, WAY - 1], F32, tag="nz")
        nc.vector.tensor_reduce(nz[:], gt0[:], X, AT.add)
        nc.vector.tensor_scalar(nz[:], nz[:], 1.0, None, AT.max)
        thr = thrp.tile([WAY, WAY - 1], F32, tag="thr")
        nc.vector.reciprocal(thr[:], nz[:])
        nc.vector.tensor_tensor(thr[:], thr[:], rsum[:], AT.mult)
        mask_slots = thrbig.tile([WAY, WAY - 1, S], F32, tag="mask_slots")
        nc.vector.tensor_tensor(
            mask_slots[:], rec_slots[:],
            thr[:, :, None].to_broadcast((WAY, WAY - 1, S)), AT.is_lt)
        maskf = thrbig.tile([WAY, SALL], F32, tag="maskf")
        nc.vector.memset(maskf[:], 0.0)
        for c in range(WAY):
            for k in range(WAY - 1):
                oc = k if k < c else k + 1
                nc.sync.dma_start(maskf[c:c + 1, oc * S:(oc + 1) * S],
                                  mask_slots[c:c + 1, k])
        nc.vector.tensor_reduce(msum[:], maskf[:], X, AT.add)
        nc.vector.tensor_scalar(msum[:], msum[:], 1.0, None, AT.max)
        # msum -> row layout [1, WAY] for per-class ACT scale in phase 4
        nc.sync.dma_start(msum_dram[:, :], msum[:])
        if dbg:
            nc.sync.dma_start(dbg["mask"].ap(), maskf[:])
        mb = thrbig.tile([WAY, SALL], BF16, tag="mb")
        nc.vector.tensor_copy(mb[:], maskf[:])
        nc.sync.dma_start(mask_dram[:, :], mb[:])

    # ================= Phase 4: contrast row sums + finals =================
    with tc.tile_pool(name="p4", bufs=2) as p4, \
         tc.tile_pool(name="p4m", bufs=1) as p4m, \
         tc.tile_pool(name="finps", bufs=2, space="PSUM") as finps:
        sel_sb = p4m.tile([128, ITILES, NQ], F32, tag="sel_sb")
        nc.sync.dma_start(sel_sb[:], sel_d.rearrange("t p q -> p t q"))
        mask_bc = p4m.tile([128, WAY, SALL], BF16, tag="mask_bc")
        for c in range(WAY):
            nc.sync.dma_start(
                mask_bc[:, c],
                _ap(mask_dram.tensor, mask_dram.offset + c * SALL,
                    [(0, 128), (1, SALL)]))
        scratch = p4m.tile([128, SALL], BF16, tag="scr")
        msum_row = p4m.tile([1, WAY], F32, tag="msum_row")
        nc.sync.dma_start(msum_row[:], _ap(msum_dram.tensor, msum_dram.offset,
                                           [(0, 1), (1, WAY)]))
        sc_row = p4m.tile([1, WAY], F32, tag="sc_row")
        nc.vector.reciprocal(sc_row[:], msum_row[:])
        nc.vector.tensor_scalar(sc_row[:], sc_row[:], 1.0 / 180.0, None, AT.mult)
        for it in range(ITILES):
            dbfl = p4.tile([128, SALL], BF16, tag="dbf_l")
            nc.sync.dma_start(dbfl[:], dbf_dram[it * 128:(it + 1) * 128])
            for c in range(WAY):
                nc.vector.scalar_tensor_tensor(
                    scratch[:], dbfl[:], 1.0, mask_bc[:, c],
                    op0=AT.mult, op1=AT.mult,
                    accum_out=rowacc[:, it, c:c + 1])
        for c in range(WAY):
            ps = finps.tile([1, NQ], F32, tag="dm_ps")
            for it in range(ITILES):
                nc.tensor.matmul(ps[:], dmax_all[:, it, c:c + 1], sel_sb[:, it],
                                 start=(it == 0), stop=(it == ITILES - 1))
            nc.scalar.activation(dmaxq[:, c], ps[:], ACTF.Copy, scale=1.0 / 45.0)
            ps2 = finps.tile([1, NQ], F32, tag="ct_ps")
            for it in range(ITILES):
                nc.tensor.matmul(ps2[:], rowacc[:, it, c:c + 1], sel_sb[:, it],
                                 start=(it == 0), stop=(it == ITILES - 1))
            nc.scalar.mul(ctq[:, c], ps2[:], sc_row[:, c:c + 1])

        for c in range(WAY):
            ssum = p4.tile([1, NQ], F32, tag="ssum")
            nc.vector.tensor_tensor(ssum[:], dmaxq[:, c], ctq[:, c], AT.add)
            rcp = p4.tile([1, NQ], F32, tag="rcp")
            nc.vector.reciprocal(rcp[:], ssum[:])
            lg = p4.tile([1, NQ], F32, tag="lg")
            nc.vector.tensor_tensor(lg[:], dmaxq[:, c], rcp[:], AT.mult)
            nc.sync.dma_start(_ap(out_d, c, [(0, 1), (WAY, NQ)]), dmaxq[:, c])
            nc.sync.dma_start(_ap(out_d, NQ * WAY + c, [(0, 1), (WAY, NQ)]), lg[:])

    ph2.release()
    persist.release()
    dram.release()


# ---------------- host side ----------------

def _sel_host():
    sel = np.zeros((ITILES, 128, NQ), np.float32)
    for i in range(R):
        sel[i // 128, i % 128, i % NQ] = 1.0
    return sel


def _prep_inputs(support_set, queries, support_labels, W, b):
    import ml_dtypes
    bf16 = ml_dtypes.bfloat16
    support_set = np.asarray(support_set, dtype=np.float32)
    queries = np.asarray(queries, dtype=np.float32)
    labels = np.asarray(support_labels).astype(np.int64)
    W = np.asarray(W, dtype=np.float32)
    b = np.asarray(b, dtype=np.float32)
    assert not np.any(b), "kernel built without bias support (reference b==0)"
    order = np.argsort(labels, kind="stable")
    support_sorted = support_set[order]
    # wT [KC, 128, DOUT]: wT[kc, p, d] = W[d, kc*128+p]
    wT = np.ascontiguousarray(
        W.T.astype(bf16).reshape(KC, 128, DOUT))
    # sT [128, 16, f*80+u]: sT[p, kc2, f*80+u] = support_sorted[u, f, kc2*128+p]
    sbf = support_sorted.astype(bf16)           # [80, 10, 2048]
    sT = np.ascontiguousarray(
        sbf.reshape(80, SEQ_LEN, 16, 128).transpose(3, 2, 1, 0)
           .reshape(128, 16, SEQ_LEN * 80))
    qbf_all = queries.astype(bf16)              # [320, 10, 2048]
    sel = _sel_host()
    padv = np.zeros((128, 1), np.float32)
    padv[8:] = 1.0e30
    out = []
    for k in range(NCORES):
        qk = qbf_all[k * NQ:(k + 1) * NQ]       # [40, 10, 2048]
        qT = np.ascontiguousarray(
            qk.reshape(NQ, SEQ_LEN, 16, 128).transpose(3, 2, 1, 0)
              .reshape(128, 16, SEQ_LEN * NQ))
        out.append({
            "qT": qT,
            "sT": sT,
            "wT": wT,
            "b": b,
            "sel": sel,
            "padv": padv,
        })
    return out


def kernel(**inputs):
    per_core = _prep_inputs(**inputs)
    if "nc" not in _CACHE:
        _CACHE["nc"] = build(debug=bool(os.environ.get("BIMACL_DEBUG")))
    nc = _CACHE["nc"]
    res = run_bass_kernel_spmd(nc, per_core, core_ids=list(range(NCORES)))
    _CACHE["last_results"] = res
    full = np.concatenate([res.results[k]["out"] for k in range(NCORES)], axis=1)
    return np.ascontiguousarray(full.astype(np.float32))



# revision 26
# speedup vs baseline: 3.8981x; 3.8981x over previous
"""Trainium2 Bass kernel for nn_CNN_BiMACL_31860067401819 (retrieval_knn).

v2 rewrite. Self-contained: hardcodes all shapes/sharding. kernel(**inputs)
accepts FULL inputs keyed as in setup_inputs(), returns FULL [2, 320, 5] f32.

Key structure (per core k of 8):
 - queries are data-parallel (40/core); support-side work (embeddings, S-S
   distance matrix) is sharded 8x and exchanged with AllGathers.
 - embeddings use the per-frame half-embedding trick: emb(q,(f1,f2)) =
   relu(A[q,f1]+B[q,f2]) with A=W1@q[f], B=W2@q[f] -> 4.5x fewer matmul flops.
 - all big matmuls run fp8(e4m3) with DoubleRow packing (2 k-chunks/pass).
 - distance epilogue d2 = qn + sn - 2*dot is fused: the packed 10th k-chunk
   carries (-sn/256) against an all-ones q chunk; qn/pn enter as ACT bias, so
   PSUM eviction is a single ACT op (Sqrt for D, Identity for S-S).
 - rec counting (cd > ave) runs on the Pool engine (gpsimd stt) over rows
   DMA-gathered from the AllGathered S-S matrix; partition_all_reduce + a tiny
   cross-core AllReduce produce the global rec.
 - the contrast masked row-sums run on PE: D tiles are PE-transposed into an
   fp8 dbfT [j, i] resident in SBUF, phase 4 is then 60 small matmuls with the
   (post-AllReduce) mask as stationary operand.
"""
import os
from itertools import combinations

import numpy as np

import concourse.bass as bass
import concourse.bass_isa as bass_isa
import concourse.tile as tile
from concourse import bacc, mybir
from concourse.bass_utils import run_bass_kernel_spmd

# ---- static problem config ----
WAY, SHOT, SEQ_LEN, TSS = 5, 16, 10, 2
DIN, DOUT = 2048, 1152
N_QUERIES = 320
T = 45
S = SHOT * T                  # 720
SALL = WAY * S                # 3600
NCORES = 8
NQ = N_QUERIES // NCORES      # 40
R = NQ * T                    # 1800 valid rows/core
RHAT = 1920                   # 48 t-slots * 40 q
ITILES = RHAT // 128          # 15
DC = DOUT // 128              # 9
DCA = DC + 3                  # +3: sn/256 bcast, correction row, zero pad
TUPLES = np.array(list(combinations(range(SEQ_LEN), TSS)), dtype=np.int32)
SH = SALL // NCORES           # 450 support cols per core
RT = [128, 128, 128, 66]      # shard row tiles
PROW = 3712                   # p row pitch (bf16 elems; bytes % 256 == 0)
NUS = 10                      # support items per core
SHP = 464                     # sloc col pitch (16B aligned for DoubleRow)
WSCALE = 1.0                  # weights stay bf16; no scaling

F32 = mybir.dt.float32
BF16 = mybir.dt.bfloat16
FP8 = mybir.dt.float8e4
U32 = mybir.dt.uint32
I16 = mybir.dt.int16

_CACHE = {}

# f1 runs: tuples t are (f1,f2) lexicographic; for fixed f1 the t-range is
# contiguous: t0(f1) = sum_{j<f1}(9-j), len = 9-f1.
F1T0 = [0]
for _f1 in range(9):
    F1T0.append(F1T0[-1] + (9 - _f1))


def _ap(tensor, offset, dims):
    return bass.AP(tensor=tensor, offset=offset, ap=[list(d) for d in dims])


def build(debug=False, sim1=False, stop_after=None):
    nc = bacc.Bacc(num_swdge_queues=4)
    w_d = nc.dram_tensor("wT", [2 * 16, 128, DOUT], BF16, kind="ExternalInput")
    q_d = nc.dram_tensor("qT", [128, 16, NQ * SEQ_LEN], BF16, kind="ExternalInput")
    s_d = nc.dram_tensor("sT", [128, 16, NUS * SEQ_LEN], BF16, kind="ExternalInput")
    sel_d = nc.dram_tensor("sel", [ITILES, 128, NQ], F32, kind="ExternalInput")
    padv_d = nc.dram_tensor("padv", [128, 1], F32, kind="ExternalInput")
    id_d = nc.dram_tensor("ident", [128, 128], BF16, kind="ExternalInput")
    out_d = nc.dram_tensor("out", [2, NQ, WAY], F32, kind="ExternalOutput")
    dbg = {}
    if debug:
        dbg["qemb"] = nc.dram_tensor("dbg_qemb", [128, DCA, RHAT], F32, kind="ExternalOutput")
        dbg["semb"] = nc.dram_tensor("dbg_semb", [128, DCA, PROW], F32, kind="ExternalOutput")
        dbg["snorm"] = nc.dram_tensor("dbg_snorm", [1, SALL], F32, kind="ExternalOutput")
        dbg["qnorm"] = nc.dram_tensor("dbg_qnorm", [128, ITILES], F32, kind="ExternalOutput")
        dbg["p"] = nc.dram_tensor("dbg_p", [SH, SALL], F32, kind="ExternalOutput")
        dbg["D"] = nc.dram_tensor("dbg_D", [RHAT, SALL], F32, kind="ExternalOutput")
        dbg["ave2"] = nc.dram_tensor("dbg_ave2", [128, ITILES, WAY], F32, kind="ExternalOutput")
        dbg["dmax"] = nc.dram_tensor("dbg_dmax", [128, ITILES, WAY], F32, kind="ExternalOutput")
        dbg["pos"] = nc.dram_tensor("dbg_pos", [128, ITILES, WAY], F32, kind="ExternalOutput")
        dbg["rec"] = nc.dram_tensor("dbg_rec", [WAY, 4 * S], F32, kind="ExternalOutput")
        dbg["mask"] = nc.dram_tensor("dbg_mask", [WAY, 4 * 768], F32, kind="ExternalOutput")
        dbg["ctq"] = nc.dram_tensor("dbg_ctq", [WAY, RHAT], F32, kind="ExternalOutput")

    with tile.TileContext(nc) as tc:
        _body(nc, tc, w_d, q_d, s_d, sel_d, padv_d, id_d, out_d, dbg,
              sim1, stop_after)
    nc.finalize()
    return nc


def _body(nc, tc, w_d, q_d, s_d, sel_d, padv_d, id_d, out_d, dbg,
          sim1, stop_after):
    AT = mybir.AluOpType
    ACTF = mybir.ActivationFunctionType
    X = mybir.AxisListType.X

    persist = tc.alloc_tile_pool(name="persist", bufs=1)
    dram = tc.alloc_tile_pool(name="dram", bufs=1, space="DRAM")

    # DRAM scratch
    sag_in = dram.tile([128, DC, SH], FP8, tag="sag_in")
    sag_out = dram.tile([NCORES, 128, DC, SH], FP8, tag="sag_out")
    snag_in = dram.tile([1, 512], F32, tag="snag_in")
    snag_out = dram.tile([NCORES, SH], F32, tag="snag_out")
    p_shard = dram.tile([SH, PROW], BF16, tag="p_shard")
    p_full = dram.tile([SALL, PROW], BF16, tag="p_full")
    posw_dram = dram.tile([WAY, 16, ITILES * 8], I16, tag="posw")
    cc_in = dram.tile([WAY, 4 * S], F32, tag="cc_in")
    cc_out = dram.tile([WAY, 4 * S], F32, tag="cc_out")
    mask_dram = dram.tile([WAY, 4, 768], FP8, tag="mask_dram")
    qn_dram = dram.tile([1, RHAT], F32, tag="qn_dram")

    # persistent SBUF (small tiles only; big tensors live in phase pools)
    embT = tc.alloc_tile_pool(name="embT", bufs=1)
    s_embT = embT.tile([128, DCA, PROW], FP8, tag="s_embT")
    q_embT = embT.tile([128, DCA, RHAT], FP8, tag="q_embT")
    qnorm = persist.tile([128, ITILES], F32, tag="qnorm")
    pnorm = persist.tile([128, 4], F32, tag="pnorm")
    ave2 = persist.tile([128, ITILES, WAY], F32, tag="ave2")
    dmax_all = persist.tile([128, ITILES, WAY], F32, tag="dmax_all")
    pos16 = persist.tile([128, ITILES, WAY], I16, tag="pos16")
    sel_sb = persist.tile([128, ITILES, NQ], F32, tag="sel_sb")
    padv = persist.tile([128, 1], F32, tag="padv")
    ident = persist.tile([128, 128], BF16, tag="ident")
    onesb = persist.tile([128, 1], BF16, tag="onesb")
    nc.sync.dma_start(padv[:], padv_d[:, :])
    nc.sync.dma_start(ident[:], id_d[:, :])
    nc.sync.dma_start(sel_sb[:], sel_d.rearrange("t p q -> p t q"))
    nc.vector.memset(onesb[:], 1.0)

    pools = [persist, dram, embT]

    def _stop(tag):
        if stop_after == tag:
            for p in reversed(pools):
                p.release()
            return True
        return False

    def ab_matmuls(embps, rhs_sb, ncols, dc):
        """A/B half-embedding psums for output chunk dc. Returns (psA, psB)."""
        psA = embps.tile([128, ncols], F32, tag="psA", name=f"psA{dc}")
        psB = embps.tile([128, ncols], F32, tag="psB", name=f"psB{dc}")
        for i in range(16):
            nc.tensor.matmul(psA[:], w_sb[:, i, dc * 128:(dc + 1) * 128],
                             rhs_sb[:, i, :], start=(i == 0), stop=(i == 15))
        for i in range(16):
            nc.tensor.matmul(psB[:], w_sb[:, 16 + i, dc * 128:(dc + 1) * 128],
                             rhs_sb[:, i, :], start=(i == 0), stop=(i == 15))
        return psA, psB

    # ================= Phase A: support-side shard =================
    slocp = tc.alloc_tile_pool(name="slocp", bufs=1)
    sloc = slocp.tile([128, DCA, SHP], FP8, tag="sloc")
    emb = tc.alloc_tile_pool(name="emb", bufs=1)

    w_sb = emb.tile([128, 32, DOUT], BF16, tag="w_sb")
    nc.sync.dma_start(w_sb[:], w_d.rearrange("kc p d -> p kc d"))

    with tc.tile_pool(name="embg", bufs=1) as embg, \
         tc.tile_pool(name="embsq", bufs=2) as embsq, \
         tc.tile_pool(name="embps", bufs=1, space="PSUM") as embps, \
         tc.tile_pool(name="snps", bufs=1, space="PSUM") as snps:
        s_sb = embg.tile([128, 16, NUS * SEQ_LEN], BF16, tag="s_sb")
        nc.sync.dma_start(s_sb[:], s_d[:, :, :])
        As = embg.tile([128, DC, NUS * SEQ_LEN], BF16, tag="As")
        Bs = embg.tile([128, DC, NUS * SEQ_LEN], BF16, tag="Bs")
        for dc in range(DC):
            psA, psB = ab_matmuls(embps, s_sb, NUS * SEQ_LEN, dc)
            nc.scalar.activation(As[:, dc], psA[:], ACTF.Copy, scale=1.0 / WSCALE)
            nc.scalar.activation(Bs[:, dc], psB[:], ACTF.Copy, scale=1.0 / WSCALE)
        # combine: sloc[:, dc, ul*45 + t] = relu(As[u,f1] + Bs[u,f2])
        scomb = embg.tile([128, SH], BF16, tag="scomb")
        for dc in range(DC):
            for f1 in range(9):
                ln = 9 - f1
                t0 = F1T0[f1]
                dst = scomb[:].rearrange("p (u t) -> p u t", t=T)[:, :, t0:t0 + ln]
                # As/Bs col = f*10+ul
                srcA = As[:, dc].rearrange("p (f u) -> p u f", u=NUS)[
                    :, :, f1:f1 + 1].to_broadcast((128, NUS, ln))
                srcB = Bs[:, dc].rearrange("p (f u) -> p u f", u=NUS)[
                    :, :, f1 + 1:]
                nc.vector.tensor_tensor(dst, srcA, srcB, AT.add)
            nc.scalar.activation(sloc[:, dc, :SH], scomb[:], ACTF.Relu)
        nc.vector.memset(sloc[:, DC], 1.0)  # incl pad cols; lhsT slices stay < SH
        nc.vector.memset(sloc[:, DC + 1], 0.0)
        nc.vector.memset(sloc[:, DC + 2], 0.0)
        nc.vector.memset(sloc[:1, DC + 1], 1.0)
        # snorm of own shard (consistent with fp8 values)
        ps_sn = snps.tile([1, SH], F32, tag="ps_sn")
        for dc in range(DC):
            sq = embsq.tile([128, SH], BF16, tag="sq", name=f"sq{dc}")
            nc.scalar.activation(sq[:], sloc[:, dc, :SH], ACTF.Square)
            nc.tensor.matmul(ps_sn[:], onesb[:], sq[:],
                             start=(dc == 0), stop=(dc == DC - 1))
        snloc = embg.tile([1, 512], F32, tag="snloc")
        nc.vector.memset(snloc[:], 0.0)
        nc.vector.tensor_copy(snloc[:, :SH], ps_sn[:])
        nc.sync.dma_start(snag_in[:, :], snloc[:])
        nc.sync.dma_start(sag_in[:, :, :], sloc[:, :DC, :SH])

        # -- AllGather s embeddings + shard norms --
        if sim1:
            for k in range(NCORES):
                nc.sync.dma_start(sag_out[k], sag_in[:, :, :])
                nc.sync.dma_start(snag_out[k:k + 1], snag_in[:, :SH])
        else:
            nc.gpsimd.collective_compute(
                "AllGather", AT.bypass, replica_groups=[list(range(NCORES))],
                ins=[sag_in[:, :, :].opt()], outs=[sag_out[:, :, :, :].opt()])
            nc.gpsimd.collective_compute(
                "AllGather", AT.bypass, replica_groups=[list(range(NCORES))],
                ins=[snag_in[:, :SH].opt()], outs=[snag_out[:, :].opt()])

        # ================= Phase B: query-side =================
        q_sb = embg.tile([128, 16, NQ * SEQ_LEN], BF16, tag="q_sb")
        nc.sync.dma_start(q_sb[:], q_d[:, :, :])
        Aq = embg.tile([128, DC, NQ * SEQ_LEN], BF16, tag="Aq")
        Bq = embg.tile([128, DC, NQ * SEQ_LEN], BF16, tag="Bq")
        for dc in range(DC):
            psA, psB = ab_matmuls(embps, q_sb, NQ * SEQ_LEN, dc)
            nc.scalar.activation(Aq[:, dc], psA[:], ACTF.Copy, scale=1.0 / WSCALE)
            nc.scalar.activation(Bq[:, dc], psB[:], ACTF.Copy, scale=1.0 / WSCALE)
        qcomb = embg.tile([128, RHAT], BF16, tag="qcomb")
        nc.vector.memset(qcomb[:, R:], 0.0)
        for dc in range(DC):
            for f1 in range(9):
                ln = 9 - f1
                t0 = F1T0[f1]
                dst = qcomb[:, t0 * NQ:(t0 + ln) * NQ].rearrange(
                    "p (f q) -> p f q", q=NQ)
                srcA = Aq[:, dc].rearrange("p (f q) -> p f q", q=NQ)[
                    :, f1:f1 + 1].to_broadcast((128, ln, NQ))
                srcB = Bq[:, dc, (f1 + 1) * NQ:(f1 + 1 + ln) * NQ].rearrange(
                    "p (f q) -> p f q", q=NQ)
                nc.vector.tensor_tensor(dst, srcA, srcB, AT.add)
            nc.scalar.activation(q_embT[:, dc], qcomb[:], ACTF.Relu)
        nc.vector.memset(q_embT[:, DC], 1.0)
        nc.vector.memset(q_embT[:, DC + 1], 0.0)
        nc.vector.memset(q_embT[:, DC + 2], 0.0)
        nc.vector.memset(q_embT[:1, DC + 1], 1.0)
        # qnorm: squares + ones-matmul partition sums -> row -> roundtrip
        qn_row = embg.tile([1, RHAT], F32, tag="qn_row")
        qnps = [embps.tile([1, 480], F32, tag=f"qn{ch}", name=f"qnps{ch}")
                for ch in range(4)]
        for dc in range(DC):
            sqf = embsq.tile([128, RHAT], BF16, tag="sqf", name=f"sqf{dc}")
            nc.scalar.activation(sqf[:], q_embT[:, dc], ACTF.Square)
            for ch in range(4):
                nc.tensor.matmul(qnps[ch][:], onesb[:],
                                 sqf[:, ch * 480:(ch + 1) * 480],
                                 start=(dc == 0), stop=(dc == DC - 1))
        for ch in range(4):
            nc.vector.tensor_copy(qn_row[:, ch * 480:(ch + 1) * 480],
                                  qnps[ch][:])
        nc.sync.dma_start(qn_dram[:, :], qn_row[:])
        nc.sync.dma_start(qnorm[:], _ap(qn_dram.tensor, qn_dram.offset,
                                        [(1, 128), (128, ITILES)]))

    emb.release()

    # ================= Phase C: assemble s_embT =================
    with tc.tile_pool(name="asm", bufs=1) as asm:
        nc.vector.memset(s_embT[:, :, SALL:], 0.0)
        for k in range(NCORES):
            nc.sync.dma_start(s_embT[:, :DC, k * SH:(k + 1) * SH], sag_out[k])
        sn_row = asm.tile([1, SALL], F32, tag="sn_row")
        nc.sync.dma_start(sn_row[:], _ap(snag_out.tensor, snag_out.offset,
                                         [(0, 1), (1, SALL)]))
        # chunk 9: uniform fp8(-sn/256) on all 128 partitions; chunk 10 row 0
        # carries the exact correction c = -sn/2 - 128*fp8(-sn/256) (cancels
        # the correlated fp8 rounding); chunk 11 is zero padding for pairing.
        sn8 = asm.tile([1, SALL], FP8, tag="sn8")
        nc.scalar.activation(sn8[:], sn_row[:], ACTF.Copy, scale=-1.0 / 256.0)
        nc.gpsimd.partition_broadcast(s_embT[:, DC, :SALL], sn8[:])
        crow = asm.tile([1, SALL], F32, tag="crow")
        nc.vector.tensor_scalar(crow[:], sn8[:], -128.0, None, AT.mult)
        nc.vector.scalar_tensor_tensor(crow[:], sn_row[:], -0.5, crow[:],
                                       op0=AT.mult, op1=AT.add)
        nc.vector.memset(s_embT[:, DC + 1, :SALL], 0.0)
        nc.vector.memset(s_embT[:, DC + 2, :SALL], 0.0)
        nc.vector.tensor_copy(s_embT[:1, DC + 1, :SALL], crow[:])
        nc.sync.dma_start(pnorm[:], _ap(snag_in.tensor, snag_in.offset,
                                        [(1, 128), (128, 4)]))
        if dbg:
            nc.sync.dma_start(dbg["snorm"].ap(), sn_row[:])
            nc.sync.dma_start(dbg["qnorm"].ap(), qnorm[:])
    if dbg:
        with tc.tile_pool(name="dbge", bufs=2) as dbge:
            for dc in range(DCA):
                dmp = dbge.tile([128, RHAT], F32, tag="dmp", name=f"dmp{dc}")
                nc.vector.tensor_copy(dmp[:], q_embT[:, dc])
                nc.sync.dma_start(dbg["qemb"][:, dc, :], dmp[:])
                dmp2 = dbge.tile([128, PROW], F32, tag="dmp2", name=f"dm2{dc}")
                nc.vector.tensor_copy(dmp2[:], s_embT[:, dc])
                nc.sync.dma_start(dbg["semb"][:, dc, :], dmp2[:])
    if _stop("emb"):
        return

    # ================= Phase D: S-S distance shard =================
    with tc.tile_pool(name="ssb", bufs=2) as ssbp, \
         tc.tile_pool(name="ssps", bufs=2, space="PSUM") as ssps:
        for rt in range(4):
            rows = RT[rt]
            p_sb = ssbp.tile([128, SALL], BF16, tag="p_sb")
            for ch in range(8):
                ps = ssps.tile([128, SH], F32, tag="ss_ps", name=f"ssps{rt}_{ch}")
                for i in range(6):
                    nc.tensor.matmul(
                        ps[:rows], sloc[:, 2 * i:2 * i + 2, rt * 128:rt * 128 + rows],
                        s_embT[:, 2 * i:2 * i + 2, ch * SH:(ch + 1) * SH],
                        start=(i == 0), stop=(i == 5),
                        perf_mode=mybir.MatmulPerfMode.DoubleRow)
                nc.scalar.activation(p_sb[:rows, ch * SH:(ch + 1) * SH], ps[:rows],
                                     ACTF.Identity, bias=pnorm[:rows, rt:rt + 1],
                                     scale=-2.0)
            nc.sync.dma_start(
                _ap(p_shard.tensor, p_shard.offset + rt * 128 * PROW,
                    [(PROW, rows), (1, SALL)]),
                p_sb[:rows])
            if dbg:
                pd = ssbp.tile([128, SALL], F32, tag="pdmp")
                nc.vector.tensor_copy(pd[:rows], p_sb[:rows])
                nc.sync.dma_start(dbg["p"][rt * 128:rt * 128 + rows, :], pd[:rows])
    slocp.release()
    if sim1:
        for k in range(NCORES):
            nc.sync.dma_start(p_full[k * SH:(k + 1) * SH, :], p_shard[:, :])
    else:
        nc.gpsimd.collective_compute(
            "AllGather", AT.bypass, replica_groups=[list(range(NCORES))],
            ins=[p_shard[:, :].opt()], outs=[p_full[:, :].opt()])

    if _stop("ss"):
        return

    # ================= Phase E: D loop =================
    big = tc.alloc_tile_pool(name="big", bufs=1)
    pools.append(big)
    dbfT = big.tile([128, 30, RHAT], FP8, tag="dbfT")
    acc9 = big.tile([128, 960], BF16, tag="acc9")
    maskT = big.tile([128, 30, 16], FP8, tag="maskT")
    # zero each class's last j-tile up front: transposes only write rows
    # 0:80 there and the phase-4 matmul must see zeros (not NaN) above.
    nc.vector.memset(
        dbfT[:].rearrange("p (c j) r -> p c j r", j=6)[:, :, 5], 0.0)
    with tc.tile_pool(name="dl", bufs=5) as dl, \
         tc.tile_pool(name="dred", bufs=3) as dred, \
         tc.tile_pool(name="dps", bufs=2, space="PSUM") as dps, \
         tc.tile_pool(name="tps", bufs=2, space="PSUM") as tps, \
         tc.tile_pool(name="recps", bufs=1, space="PSUM") as recps, \
         tc.tile_pool(name="cdp", bufs=1) as cdp, \
         tc.tile_pool(name="cdg", bufs=3) as cdg, \
         tc.tile_pool(name="cmpp", bufs=4) as cmpp, \
         tc.tile_pool(name="recp", bufs=1) as recp:
        ITG = [(0, 4), (4, 8), (8, 12), (12, 15)]
        gst = {}

        def setup_gather(c):
            nc.sync.dma_start(
                _ap(posw_dram.tensor, posw_dram.offset + c * 16 * ITILES * 8,
                    [(1, 8), (ITILES * 8, 16), (8, ITILES)]),
                pos16[:, :, c])
            idxs = cdp.tile([128, ITILES * 8], I16, tag="idxs",
                            name=f"idxs{c}")
            nc.sync.dma_start(
                idxs[:],
                _ap(posw_dram.tensor, posw_dram.offset + c * 16 * ITILES * 8,
                    [(0, 8), (ITILES * 8, 16), (1, ITILES * 8)]))
            spans = []
            if c > 0:
                spans.append((0, 0, c * S))
            if c < WAY - 1:
                spans.append((c * S, (c + 1) * S, (4 - c) * S))
            rcps = [recps.tile([1, 480], F32, tag=f"rc{ch}", name=f"rc{c}_{ch}")
                    for ch in range(4)]
            gst[c] = (idxs, spans, rcps)

        def gather_unit(c, g):
            idxs, spans, rcps = gst[c]
            cd = cdg.tile([128, 1, PROW], BF16, tag="cd", name=f"cd{c}_{g}")
            nc.gpsimd.dma_gather(
                cd[:], p_full[:, :], idxs[:, g * 8:(g + 1) * 8],
                128, 128, PROW, queue_num=g % 4)
            cmp = cmpp.tile([128, 4 * S], BF16, tag="cmp", name=f"cm{c}_{g}")
            eng = nc.vector
            for (d0, s0, ln) in spans:
                eng.tensor_scalar(cmp[:, d0:d0 + ln],
                                  cd[:, 0, s0:s0 + ln],
                                  ave2[:, g, c:c + 1], None, AT.is_gt)
            for ch in range(4):
                nc.tensor.matmul(rcps[ch][:], onesb[:],
                                 cmp[:, ch * 480:(ch + 1) * 480],
                                 start=(g == 0), stop=(g == ITILES - 1))
            if g == 0:
                nc.vector.tensor_copy(acc9[:], cmp[:, 1920:])
            else:
                aeng = nc.gpsimd if (g % 2) else nc.vector
                aeng.tensor_tensor(acc9[:], acc9[:], cmp[:, 1920:], AT.add)

        def finish_gather(c):
            idxs, spans, rcps = gst.pop(c)
            rec_row = recp.tile([1, 4 * S], F32, tag="rec_row",
                                name=f"rr{c}")
            for ch in range(4):
                nc.vector.tensor_copy(rec_row[:, ch * 480:(ch + 1) * 480],
                                      rcps[ch][:])
            rfo = recp.tile([128, 960], F32, tag="rfo", name=f"rfo{c}")
            nc.gpsimd.partition_all_reduce(rfo[:], acc9[:], 128,
                                           bass_isa.ReduceOp.add)
            nc.vector.tensor_copy(rec_row[:, 1920:], rfo[:1])
            nc.sync.dma_start(cc_in[c:c + 1, :], rec_row[:])

        for c in range(WAY):
            d_bfs = {}
            for g0, g1 in ITG:
                for it in range(g0, g1):
                    d_bf = dl.tile([128, S], BF16, tag="d_bf", name=f"dbf{c}_{it}")
                    d_bfs[it] = d_bf
                    for sc in range(2):
                        ps = dps.tile([128, 360], F32, tag="d_ps",
                                      name=f"dps{c}_{it}_{sc}")
                        for i in range(6):
                            nc.tensor.matmul(
                                ps[:], q_embT[:, 2 * i:2 * i + 2, it * 128:(it + 1) * 128],
                                s_embT[:, 2 * i:2 * i + 2,
                                       c * S + sc * 360:c * S + (sc + 1) * 360],
                                start=(i == 0), stop=(i == 5),
                                perf_mode=mybir.MatmulPerfMode.DoubleRow)
                        nc.scalar.activation(d_bf[:, sc * 360:(sc + 1) * 360],
                                             ps[:], ACTF.Sqrt,
                                             bias=qnorm[:, it:it + 1], scale=-2.0)
                    if dbg:
                        dd = dred.tile([128, S], F32, tag="ddmp")
                        nc.vector.tensor_copy(dd[:], d_bf[:])
                        nc.sync.dma_start(
                            dbg["D"][it * 128:(it + 1) * 128, c * S:(c + 1) * S],
                            dd[:])
                    # reductions (reference quirk: groups are s%16)
                    m16 = dred.tile([128, 16], F32, tag="m16")
                    nc.vector.tensor_reduce(
                        m16[:], d_bf[:].rearrange("p (a b) -> p b a", b=16),
                        X, AT.max)
                    av = dred.tile([128, 1], F32, tag="av")
                    nc.vector.tensor_reduce(av[:], m16[:], X, AT.add)
                    if it == ITILES - 1:
                        nc.vector.scalar_tensor_tensor(
                            av[:], av[:], 1.0 / 16.0, padv[:],
                            op0=AT.mult, op1=AT.add)
                    else:
                        nc.vector.tensor_scalar(av[:], av[:], 1.0 / 16.0, None,
                                                AT.mult)
                    nc.vector.tensor_tensor(ave2[:, it, c:c + 1], av[:], av[:],
                                            AT.mult)
                    nc.vector.tensor_reduce(dmax_all[:, it, c:c + 1], m16[:],
                                            X, AT.max)
                    mx8 = dred.tile([128, 8], BF16, tag="mx8")
                    ix8 = dred.tile([128, 8], U32, tag="ix8")
                    nc.vector.max(mx8[:], d_bf[:])
                    nc.vector.max_index(ix8[:], mx8[:], d_bf[:])
                    posf = dred.tile([128, 1], F32, tag="posf")
                    nc.vector.tensor_scalar(posf[:], ix8[:, 0:1], float(c * S),
                                            None, AT.add)
                    nc.vector.tensor_copy(pos16[:, it, c:c + 1], posf[:])
                    # pipelined: previous class's gather/compare/rec unit
                    if c >= 1:
                        gather_unit(c - 1, it)
                # transposes for this it-group -> dbfT
                glen = g1 - g0
                for jt in range(6):
                    jn = 128 if jt < 5 else 80
                    psT = tps.tile([128, 512], BF16, tag="psT",
                                   name=f"psT{c}_{g0}_{jt}")
                    for it in range(g0, g1):
                        nc.tensor.transpose(
                            psT[:jn, (it - g0) * 128:(it - g0 + 1) * 128],
                            d_bfs[it][:, jt * 128:jt * 128 + jn], ident[:])
                    nc.scalar.activation(
                        dbfT[:jn, c * 6 + jt, g0 * 128:g1 * 128],
                        psT[:jn, :glen * 128], ACTF.Copy)
            if c >= 1:
                finish_gather(c - 1)
            setup_gather(c)
        for g in range(ITILES):
            gather_unit(WAY - 1, g)
        finish_gather(WAY - 1)

    if dbg:
        nc.sync.dma_start(dbg["ave2"].ap(), ave2[:])
        nc.sync.dma_start(dbg["dmax"].ap(), dmax_all[:])
        with tc.tile_pool(name="dbgp", bufs=1) as dbgp:
            pf = dbgp.tile([128, ITILES, WAY], F32, tag="pf")
            nc.vector.tensor_copy(pf[:], pos16[:])
            nc.sync.dma_start(dbg["pos"].ap(), pf[:])

    if _stop("dloop"):
        return

    # ================= AllReduce rec =================
    if sim1:
        nc.sync.dma_start(cc_out[:, :], cc_in[:, :])
    else:
        nc.gpsimd.collective_compute(
            "AllReduce", AT.add, replica_groups=[list(range(NCORES))],
            ins=[cc_in[:, :].opt()], outs=[cc_out[:, :].opt()])

    # ================= Phase F: thr/mask + contrast + finals =================
    nc.vector.memset(maskT[:], 0.0)
    with tc.tile_pool(name="thr", bufs=1) as thr, \
         tc.tile_pool(name="fin", bufs=2) as fin, \
         tc.tile_pool(name="fps", bufs=2, space="PSUM") as fps:
        rec_sb = thr.tile([WAY, 4, S], F32, tag="rec_sb")
        nc.sync.dma_start(rec_sb[:], cc_out.rearrange("c (k s) -> c k s", k=4))
        if dbg:
            nc.sync.dma_start(dbg["rec"].ap(), cc_out[:, :])
        rsum = thr.tile([WAY, 4], F32, tag="rsum")
        nc.vector.tensor_reduce(rsum[:], rec_sb[:], X, AT.add)
        gt0 = thr.tile([WAY, 4, S], BF16, tag="gt0")
        nc.vector.tensor_scalar(gt0[:], rec_sb[:], 0.0, None, AT.is_gt)
        nz = thr.tile([WAY, 4], F32, tag="nz")
        nc.vector.tensor_reduce(nz[:], gt0[:], X, AT.add)
        nc.vector.tensor_scalar(nz[:], nz[:], 1.0, None, AT.max)
        thr_t = thr.tile([WAY, 4], F32, tag="thr_t")
        nc.vector.reciprocal(thr_t[:], nz[:])
        nc.vector.tensor_tensor(thr_t[:], thr_t[:], rsum[:], AT.mult)
        mask8 = thr.tile([WAY, 4, 768], FP8, tag="mask8")
        nc.vector.memset(mask8[:], 0.0)
        msl = mask8[:, :, :S]
        nc.vector.tensor_tensor(
            msl, rec_sb[:], thr_t[:, :, None].to_broadcast((WAY, 4, S)),
            AT.is_lt)
        msum = thr.tile([WAY, 1], F32, tag="msum")
        nc.vector.tensor_reduce(
            msum[:], mask8[:].rearrange("c k s -> c (k s)"), X, AT.add)
        nc.vector.tensor_scalar(msum[:], msum[:], 1.0, None, AT.max)
        scal = thr.tile([WAY, 1], F32, tag="scal")
        nc.vector.tensor_scalar(scal[:], msum[:], 180.0, None, AT.mult)
        nc.vector.reciprocal(scal[:], scal[:])
        nc.sync.dma_start(mask_dram[:, :, :], mask8[:])
        if dbg:
            md = thr.tile([WAY, 4, 768], F32, tag="mdmp")
            nc.vector.tensor_copy(md[:], mask8[:])
            nc.sync.dma_start(dbg["mask"].ap(), md[:])
        # maskT[j=(c',t,p), c] = mask[c, k(c,c'), t*128+p]; own class stays 0
        for c in range(WAY):
            for cp in range(WAY):
                if cp == c:
                    continue
                k = cp if cp < c else cp - 1
                nc.sync.dma_start(
                    maskT[:, cp * 6:(cp + 1) * 6, c],
                    _ap(mask_dram.tensor,
                        mask_dram.offset + (c * 4 + k) * 768,
                        [(1, 128), (128, 6)]))
        # contrast sums on PE over transposed D
        ctq_sb = fin.tile([WAY, RHAT], F32, tag="ctq_sb")
        for ic in range(4):
            ps = fps.tile([16, 480], F32, tag="ct_ps", name=f"ctps{ic}")
            for tpair in range(15):
                nc.tensor.matmul(
                    ps[:], maskT[:, 2 * tpair:2 * tpair + 2, :],
                    dbfT[:, 2 * tpair:2 * tpair + 2, ic * 480:(ic + 1) * 480],
                    start=(tpair == 0), stop=(tpair == 14),
                    perf_mode=mybir.MatmulPerfMode.DoubleRow)
            nc.scalar.activation(ctq_sb[:, ic * 480:(ic + 1) * 480], ps[:WAY],
                                 ACTF.Copy, scale=scal[:, 0:1])
        if dbg:
            nc.sync.dma_start(dbg["ctq"].ap(), ctq_sb[:])
        ctq40 = fin.tile([WAY, NQ], F32, tag="ctq40")
        nc.vector.tensor_reduce(
            ctq40[:],
            ctq_sb[:].rearrange("c (t q) -> c q t", q=NQ)[:, :, :T],
            X, AT.add)
        # dist_max per query
        psd = fps.tile([WAY, NQ], F32, tag="dm_ps")
        for it in range(ITILES):
            nc.tensor.matmul(psd[:], dmax_all[:, it, :], sel_sb[:, it],
                             start=(it == 0), stop=(it == ITILES - 1))
        dmq = fin.tile([WAY, NQ], F32, tag="dmq")
        nc.scalar.activation(dmq[:], psd[:], ACTF.Copy, scale=1.0 / T)
        ssum = fin.tile([WAY, NQ], F32, tag="ssum")
        nc.vector.tensor_tensor(ssum[:], dmq[:], ctq40[:], AT.add)
        rcp = fin.tile([WAY, NQ], F32, tag="rcp")
        nc.vector.reciprocal(rcp[:], ssum[:])
        lg = fin.tile([WAY, NQ], F32, tag="lg")
        nc.vector.tensor_tensor(lg[:], dmq[:], rcp[:], AT.mult)
        nc.sync.dma_start(_ap(out_d, 0, [(1, WAY), (WAY, NQ)]), dmq[:])
        nc.sync.dma_start(_ap(out_d, NQ * WAY, [(1, WAY), (WAY, NQ)]), lg[:])

    for p in reversed(pools):
        p.release()


# ---------------- host side ----------------

def _sel_host():
    sel = np.zeros((ITILES, 128, NQ), np.float32)
    for i in range(R):
        sel[i // 128, i % 128, i % NQ] = 1.0
    return sel


def _prep_inputs(support_set, queries, support_labels, W, b):
    import ml_dtypes
    fp8 = ml_dtypes.float8_e4m3
    bf16 = ml_dtypes.bfloat16
    support_set = np.asarray(support_set, dtype=np.float32)
    queries = np.asarray(queries, dtype=np.float32)
    labels = np.asarray(support_labels).astype(np.int64)
    W = np.asarray(W, dtype=np.float32)
    b = np.asarray(b, dtype=np.float32)
    assert not np.any(b), "kernel built without bias support (reference b==0)"
    order = np.argsort(labels, kind="stable")
    support_sorted = support_set[order]            # [80, 10, 2048]

    # wT [32, 128, DOUT]: chunk h*16+kc -> W[d, h*2048 + kc*128 + p]
    wT = np.ascontiguousarray(W.T.astype(bf16).reshape(32, 128, DOUT))
    ident = np.eye(128, dtype=bf16)
    sel = _sel_host()
    padv = np.zeros((128, 1), np.float32)
    padv[8:] = 1.0e18
    qf8 = queries.astype(bf16)                     # [320, 10, 2048]
    sf8 = support_sorted.astype(bf16)              # [80, 10, 2048]
    out = []
    for k in range(NCORES):
        qk = qf8[k * NQ:(k + 1) * NQ]              # [40, 10, 2048]
        qT = np.ascontiguousarray(
            qk.reshape(NQ, SEQ_LEN, 16, 128).transpose(3, 2, 1, 0)
              .reshape(128, 16, SEQ_LEN * NQ))
        sk = sf8[k * NUS:(k + 1) * NUS]            # [10, 10, 2048]
        sT = np.ascontiguousarray(
            sk.reshape(NUS, SEQ_LEN, 16, 128).transpose(3, 2, 1, 0)
              .reshape(128, 16, SEQ_LEN * NUS))
        out.append({
            "qT": qT, "sT": sT, "wT": wT,
            "sel": sel, "padv": padv, "ident": ident,
        })
    return out


def kernel(**inputs):
    per_core = _prep_inputs(**inputs)
    if "nc" not in _CACHE:
        _CACHE["nc"] = build(debug=bool(os.environ.get("BIMACL_DEBUG")))
    nc = _CACHE["nc"]
    res = run_bass_kernel_spmd(nc, per_core, core_ids=list(range(NCORES)))
    _CACHE["last_results"] = res
    full = np.concatenate([res.results[k]["out"] for k in range(NCORES)], axis=1)
    return np.ascontiguousarray(full.astype(np.float32))
